# revision 15
# baseline (speedup 1.0000x reference)
"""Trainium2 Bass kernel for AtomTypeGNN message passing.

Computation (reference):
    adj_exp[m, f] = sum_n dist_adj[m, n] * dist_exp[m, n, f]          # [N, F]
    feat[m, k]    = sum_{f,h} adj_exp[m, f] * W[f, h, k] * emb[m, h]  # [N, K]
    out           = softplus(feat) + b                                # [N, K]

Sharding: rows m across 8 cores (256 rows each); W/b replicated. No
cross-core communication needed.

Inputs are cast to fp16 on the host (halves the dominant dist_exp DMA
stream and avoids fp32 LOW/HIGH matmul splitting); all accumulation
stays fp32 in PSUM.

Per-core device algorithm (m-blocks of 128):
  Step 1 on the TensorEngine: for each m and each 128-wide n-chunk j,
    psum_adjT[0:64, m] += E_chunk(m,j)[128n, 64f].T @ A_col(m,j)[128n, 1]
  E[m] is DMA'd as [128, 16*64] (partition p holds n in [16p, 16p+16)),
  and A arrives host-prearranged so column (m*16+j) is A[m, 16p+j].
  PSUM accumulation uses memset + start=False so per-column groups are
  order independent.
  Step 2: O_T[(f,h), m] = adj_exp[m, f] * emb[m, h] built per-f via
  tensor_scalar (per-partition scalar) + PE transpose, then one batched
  matmul against W reshaped [F*H, K] accumulating over 64 fh-chunks.
  Epilogue: stable softplus (relu + ln(1+exp(-|x|)) on ScalarE) plus
  per-partition bias, output stored transposed [K, m] and untransposed
  on the host.
"""

import numpy as np

import concourse.bass as bass
import concourse.mybir as mybir
import concourse.tile as tile
from concourse import bacc
from concourse.bass_utils import run_bass_kernel_spmd
from concourse.masks import make_identity

F32 = mybir.dt.float32
F16 = mybir.dt.float16
NP_F16 = np.float16

N_CORES = 8
NA = 2048          # total atoms (n dimension)
F = 64             # dist_exp_size
H = 128            # atom_emb_size
K = 256            # hidden_size
M_SH = NA // N_CORES   # 256 rows per core
M_BLK = 128            # m-block (PSUM column count)


def build(m_sh=M_SH, na=NA, mgrp=8, e_bufs=4):
    """Build the per-core program. m_sh: rows per core, na: contraction size."""
    jj = na // 128            # n-chunks per m
    n_mb = m_sh // M_BLK      # m-blocks
    kh_n = K // 128           # output k halves

    nc = bacc.Bacc(None, target_bir_lowering=False)
    de = nc.declare_dram_parameter("dist_exp", [m_sh, na, F], F16, isOutput=False)
    a_send = nc.declare_dram_parameter("a_send", [128, m_sh * jj], F16, isOutput=False)
    emb = nc.declare_dram_parameter("emb", [m_sh, H], F16, isOutput=False)
    w2 = nc.declare_dram_parameter("w2", [F * H, K], F16, isOutput=False)
    bias = nc.declare_dram_parameter("bias", [128, kh_n], F32, isOutput=False)
    out = nc.declare_dram_parameter("out", [K, m_sh], F32, isOutput=True)

    # [128, m, jj*64]: partition p holds E[m, p*jj + j, f] at offset j*64+f
    de_r = de.rearrange("M (p j) f -> p M (j f)", p=128)
    # [128, c, K]: partition p holds w2[c*128 + p, :]
    w2_r = w2.rearrange("(c p) n -> p c n", p=128)
    emb_r = emb.rearrange("(b p) h -> p b h", p=128)

    with tile.TileContext(nc) as tc:
        with (
            tc.tile_pool(name="const", bufs=1) as cpool,
            tc.tile_pool(name="epool", bufs=e_bufs) as epool,
            tc.tile_pool(name="ot", bufs=1) as otpool,
            tc.tile_pool(name="tmp", bufs=3) as tmppool,
            tc.tile_pool(name="small", bufs=2) as smallpool,
            tc.tile_pool(name="outp", bufs=2) as outpool,
            tc.tile_pool(name="ps_adj", bufs=1, space="PSUM") as ps_adj_pool,
            tc.tile_pool(name="ps_t", bufs=2, space="PSUM") as ps_t_pool,
            tc.tile_pool(name="ps_f", bufs=2, space="PSUM") as ps_f_pool,
        ):
            # constants
            a_sb = cpool.tile([128, m_sh * jj], F16)
            nc.scalar.dma_start(a_sb[:], a_send[:])
            emb_sb = cpool.tile([128, n_mb, H], F16)
            nc.scalar.dma_start(emb_sb[:], emb_r[:])
            w2_sb = cpool.tile([128, F * H // 128, K], F16)
            nc.scalar.dma_start(w2_sb[:], w2_r[:])
            bias_sb = cpool.tile([128, kh_n], F32)
            nc.scalar.dma_start(bias_sb[:], bias[:])
            ident = cpool.tile([128, 128], F16)
            make_identity(nc, ident[:])

            for mb in range(n_mb):
                # ---- step 1 on PE: A-column stationary (1-col LDW), E-chunk
                # moving (N=64). M=1 outputs col-tile across PSUM partitions
                # {0,32,64,96}; m goes to partition 32*(m%4), cols (m//4)*64.
                # m_loc -> PSUM partition 32*(m_loc//32), cols (m_loc%32)*F.
                # E tiles stride across the four 32-m quarters so the four
                # PSUM col-groups run concurrently.
                qn = M_BLK // 4  # m's per col-group
                psum_adj = ps_adj_pool.tile([128, qn * F], F32)
                nc.vector.memset(psum_adj[:], 0.0)
                blk_src = de_r[
                    :, mb * M_BLK : (mb + 1) * M_BLK, :
                ].rearrange("p (r q) u -> p r q u", r=4)
                for q0 in range(qn):
                    et = epool.tile([128, 4, jj * 64], F16)
                    nc.sync.dma_start(et[:], blk_src[:, :, q0, :])
                    for j in range(jj):
                        for r in range(4):
                            m_loc = r * qn + q0
                            m = mb * M_BLK + m_loc
                            prow = 32 * r
                            coff = q0 * F
                            nc.tensor.matmul(
                                psum_adj[prow : prow + 1, coff : coff + F],
                                lhsT=a_sb[:, m * jj + j : m * jj + j + 1],
                                rhs=et[:, r, j * 64 : (j + 1) * 64],
                                start=False,
                                stop=(j == jj - 1),
                                skip_group_check=True,
                                tile_position=(0, prow),
                            )

                # ---- redistribute to adj_exp [m_loc, f] via SBUF copies ----
                scratch = smallpool.tile([128, qn * F], F32, tag="scr")
                nc.vector.tensor_copy(scratch[:], psum_adj[:])
                adjexp_sb = smallpool.tile([128, F], F32, tag="adjexp")
                for r in range(4):
                    nc.sync.dma_start(
                        adjexp_sb[r * qn : (r + 1) * qn, :],
                        scratch[32 * r : 32 * r + 1, :].rearrange(
                            "o (m f) -> o m f", f=F
                        ),
                    )

                # ---- O_T[(f,h), m_loc] build: scalar-mul + PE transpose ----
                ot = otpool.tile([128, F, M_BLK], F16)
                for f in range(F):
                    tmp_o = tmppool.tile([128, H], F16)
                    nc.vector.tensor_scalar_mul(
                        tmp_o[:], emb_sb[:, mb, :], adjexp_sb[:, f : f + 1]
                    )
                    psum_o = ps_t_pool.tile([128, 128], F16, tag="tr")
                    nc.tensor.transpose(psum_o[:], tmp_o[:], ident[:])
                    nc.vector.tensor_copy(ot[:, f, :], psum_o[:])

                # ---- step 2: feat_T[k, m_loc] = sum_c w2_c.T @ OT_c ----
                for kh in range(kh_n):
                    psum_f = ps_f_pool.tile([128, M_BLK], F32)
                    for c in range(F):
                        nc.tensor.matmul(
                            psum_f[:],
                            lhsT=w2_sb[:, c, kh * 128 : (kh + 1) * 128],
                            rhs=ot[:, c, :],
                            start=(c == 0),
                            stop=(c == F - 1),
                        )
                    # stable softplus: relu(x) + ln(1 + exp(-min(|x|, 30)))
                    AF = mybir.ActivationFunctionType
                    ab = outpool.tile([128, M_BLK], F32, tag="sp_t")
                    nc.scalar.activation(ab[:], psum_f[:], AF.Abs)
                    nc.vector.tensor_scalar_min(ab[:], ab[:], 30.0)
                    nc.scalar.activation(ab[:], ab[:], AF.Exp, scale=-1.0)
                    nc.scalar.activation(ab[:], ab[:], AF.Ln, bias=1.0)
                    sp_sb = outpool.tile([128, M_BLK], F32)
                    nc.scalar.activation(sp_sb[:], psum_f[:], AF.Relu)
                    nc.vector.tensor_add(sp_sb[:], sp_sb[:], ab[:])
                    nc.vector.tensor_scalar_add(
                        sp_sb[:], sp_sb[:], bias_sb[:, kh : kh + 1]
                    )
                    nc.scalar.dma_start(
                        out[kh * 128 : (kh + 1) * 128, mb * M_BLK : (mb + 1) * M_BLK],
                        sp_sb[:],
                    )
    nc.compile()
    return nc


def prep_inputs(dist_adj, dist_exp, atom_emb, bilinear_w, bilinear_b, n_cores=N_CORES):
    """Shard + host-side layout prep. Returns in_maps for run_bass_kernel_spmd."""
    na = dist_adj.shape[1]
    m_sh = dist_adj.shape[0] // n_cores
    jj = na // 128
    f, h, k = bilinear_w.shape
    w2 = np.ascontiguousarray(bilinear_w.reshape(f * h, k)).astype(NP_F16)
    bias = np.ascontiguousarray(
        np.asarray(bilinear_b, dtype=np.float32).reshape(k // 128, 128).T
    )
    de_bf = np.asarray(dist_exp).astype(NP_F16)
    in_maps = []
    for c in range(n_cores):
        sl = slice(c * m_sh, (c + 1) * m_sh)
        a = np.asarray(dist_adj[sl], dtype=np.float32)
        # a_send[p, m*jj + j] = A[m, p*jj + j]
        a_send = np.ascontiguousarray(
            a.reshape(m_sh, 128, jj).transpose(1, 0, 2).reshape(128, m_sh * jj)
        ).astype(NP_F16)
        in_maps.append(
            {
                "dist_exp": de_bf[sl],
                "a_send": a_send,
                "emb": np.asarray(atom_emb[sl]).astype(NP_F16),
                "w2": w2,
                "bias": bias,
            }
        )
    return in_maps


_NC_CACHE = {}


def _get_nc():
    if "nc" not in _NC_CACHE:
        _NC_CACHE["nc"] = build()
    return _NC_CACHE["nc"]


def kernel(dist_adj, dist_exp, atom_emb, bilinear_w, bilinear_b):
    nc = _get_nc()
    in_maps = prep_inputs(dist_adj, dist_exp, atom_emb, bilinear_w, bilinear_b)
    res = run_bass_kernel_spmd(nc, in_maps, core_ids=list(range(N_CORES)))
    return np.concatenate([r["out"].T for r in res.results], axis=0)
